# revision 1
# baseline (speedup 1.0000x reference)
"""Trainium2 Bass kernel for the KNet-style recurrent chain (batch=1).

Strategy (memory-bound problem, ~353MB of fp32 weights):
  - The small GRU chain + small FCs (~35MB) are REPLICATED on all 8 cores.
  - FC2 (the big Kalman-gain MLP: W2a [46080,1152], W2b [576,46080]) is
    tensor-parallel: each core gets 5760 rows of W2a and the matching 5760
    columns of W2b, computes a partial y [576]; the host sums the 8 partials
    and adds b2b (the "all-reduce" done on host).
  - Every matvec y = W @ x runs on the TensorEngine in WEIGHT-MOVING form:
        out[1, N] (+)= x_chunk[K, 1].T @ W.T_chunk[K, N]
    i.e. the tiny activation chunk is the stationary operand (fast fp32
    load) and the pre-transposed weights stream as the moving operand
    (~430ns per [128, 512] fp32 block, ~611 GB/s — above the per-core HBM
    rate).  Keeping weights stationary instead costs ~350ns per [128,128]
    tile (fp32 weight load), 3x too slow.
  - Matvec outputs live in free-layout [1, M] (one partition); elementwise
    GRU math happens there; PE transpose-mode matmuls ([1,128] -> [128,1],
    ~330ns) rebuild the partition-layout [128, ceil(d/128)] tiles consumed
    as the next layer's stationary chunks.
  - PSUM accumulation: start=True clears has_written for the WHOLE target
    bank, so it is set only on the first matmul into each bank; later
    first-writes to an element overwrite because has_written=0.
"""

import sys

sys.path.insert(0, "/opt/trn_rl_repo")

import numpy as np

NCORES = 8
H = 576                      # hidden size of all three GRUs
D2_HID, D2_IN, D2_OUT = 46080, 1152, 576
MSH = D2_HID // NCORES       # 5760 rows of W2a per core
NM2 = MSH // 128             # 45 output chunks per core
STRIPE = 512                 # FC2a output stripe width
W2B_GRP = 3                  # FC2b K-blocks per DMA

F32 = np.float32


def _ncols(d):
    return (d + 127) // 128


def _nsplits(m):
    """split free dim at 512 boundaries (= PSUM bank boundaries)."""
    return [(n0, min(512, m - n0)) for n0 in range(0, m, 512)]


_CACHE = {}


class _Vec:
    """An activation vector in SBUF P-layout [128, ncols]."""

    def __init__(self, tile, d):
        self.tile = tile
        self.d = d

    def chunks(self):
        for c in range(_ncols(self.d)):
            sz = min(128, self.d - c * 128)
            yield self.tile[0:sz, c : c + 1], sz


def _build_program(dbg=False):
    import concourse.bass as bass  # noqa: F401
    from concourse import bacc, mybir
    import concourse.tile as tile

    f32 = mybir.dt.float32
    f32r = mybir.dt.float32r
    AF = mybir.ActivationFunctionType

    nc = bacc.Bacc(
        "TRN2", target_bir_lowering=False, debug=False, num_devices=NCORES
    )

    def din(name, shape, dt=f32):
        return nc.dram_tensor(name, list(shape), dt, kind="ExternalInput")

    # --- dram inputs: activation vectors ---
    d_x5 = din("x5", (24, 1), f32r)
    d_x6 = din("x6", (24, 1), f32r)
    d_obs = din("obs", (48, 1), f32r)
    d_hq = din("h_q", (128, 5), f32r)      # P-layout (matvec operand)
    d_hsig = din("h_sig", (128, 5), f32r)
    d_hs = din("h_s", (128, 5), f32r)
    d_hq_f = din("h_q_f", (1, H))    # free-layout (elementwise operand)
    d_hsig_f = din("h_sig_f", (1, H))
    d_hs_f = din("h_s_f", (1, H))

    # --- dram inputs: weights, host-stored as W.T [K, M] row-major ---
    wshapes = {
        "w5": (24, 480), "w6": (24, 480), "w7": (48, 960), "w1": (576, 576),
        "wrz_q": (1056, 1152), "win_q": (480, 576), "whn_q": (576, 576),
        "wrz_sig": (1632, 1152), "win_sig": (1056, 576), "whn_sig": (576, 576),
        "wrz_s": (2112, 1152), "win_s": (1536, 576), "whn_s": (576, 576),
        "w2a": (D2_IN, MSH), "w2b": (MSH, D2_OUT),
    }
    dw = {k: din(k, v, f32r) for k, v in wshapes.items()}

    # --- dram inputs: biases in free-layout [1, M] ---
    bshapes = {
        "b5": 480, "b6": 480, "b7": 960, "b1": H,
        "brz_q": 1152, "bin_q": H, "bhn_q": H,
        "brz_sig": 1152, "bin_sig": H, "bhn_sig": H,
        "brz_s": 1152, "bin_s": H, "bhn_s": H,
    }
    db = {k: din(k, (1, v)) for k, v in bshapes.items()}

    d_b2a = din("b2a", (1, MSH))
    d_y = nc.dram_tensor("y", [1, D2_OUT], f32, kind="ExternalOutput")

    dbg_outs = {}

    def _dbg(name, tile_ap, shape):
        if not dbg:
            return
        dt = nc.dram_tensor(f"dbg_{name}", list(shape), f32,
                            kind="ExternalOutput")
        nc.sync.dma_start(out=dt[:], in_=tile_ap.bitcast(f32))
        dbg_outs[name] = dt

    with tile.TileContext(nc) as tc:
        with (
            tc.tile_pool(name="const", bufs=1) as constp,
            tc.tile_pool(name="vecs", bufs=1) as vecp,
            tc.tile_pool(name="smallw", bufs=3) as swp,
            tc.tile_pool(name="bigw", bufs=3) as bigp,
            tc.tile_pool(name="w2bp", bufs=2) as w2bp,
            tc.tile_pool(name="ps", bufs=1, space="PSUM") as psp,
        ):
            def load_const(dram, shape, name, dt=f32):
                t = constp.tile(list(shape), dt, name=name, tag=name)
                nc.sync.dma_start(out=t, in_=dram[:])
                return t

            x5 = _Vec(load_const(d_x5, (24, 1), "t_x5", f32r), 24)
            x6 = _Vec(load_const(d_x6, (24, 1), "t_x6", f32r), 24)
            obs = _Vec(load_const(d_obs, (48, 1), "t_obs", f32r), 48)
            h_q = _Vec(load_const(d_hq, (128, 5), "t_hq", f32r), H)
            h_sig = _Vec(load_const(d_hsig, (128, 5), "t_hsig", f32r), H)
            h_s = _Vec(load_const(d_hs, (128, 5), "t_hs", f32r), H)
            hf = {
                "q": load_const(d_hq_f, (1, H), "t_hq_f"),
                "sig": load_const(d_hsig_f, (1, H), "t_hsig_f"),
                "s": load_const(d_hs_f, (1, H), "t_hs_f"),
            }
            bt = {
                k: load_const(db[k], (1, v), "t_" + k)
                for k, v in bshapes.items()
            }
            ident = constp.tile([1, 1], f32, name="ident", tag="ident")
            nc.vector.memset(ident, 1.0)

            def load_w_chunks(wname, segs, m_out):
                """DMA pre-transposed weights; yield (wt_ap, rhs, ksz)."""
                w = dw[wname]
                chunks = []
                ro = 0
                # cap tile size at ~14KB/partition
                grp = max(1, 14336 // (m_out * 4))
                for v in segs:
                    nb, tail = v.d // 128, v.d % 128
                    rhs_cols = list(v.chunks())
                    for g0 in range(0, nb, grp):
                        gn = min(grp, nb - g0)
                        wt = swp.tile([128, gn, m_out], f32r, tag="sw",
                                      name=f"w_{wname}_{ro}f{g0}", bufs=3)
                        nc.sync.dma_start(
                            out=wt,
                            in_=w[ro + g0 * 128 : ro + (g0 + gn) * 128,
                                  :].rearrange("(b p) m -> p b m", p=128),
                        )
                        for b in range(gn):
                            chunks.append(
                                (wt[:, b, :], rhs_cols[g0 + b][0], 128)
                            )
                    if tail:
                        wtt = swp.tile([tail, m_out], f32r, tag="sw",
                                       name=f"w_{wname}_{ro}t", bufs=3)
                        nc.sync.dma_start(
                            out=wtt, in_=w[ro + nb * 128 : ro + v.d, :]
                        )
                        chunks.append((wtt, rhs_cols[nb][0], tail))
                    ro += v.d
                return chunks

            def matvec_f(wname, segs, m_out, bias_tile, act, out_name,
                         psum_tag, psum_bufs, out_tag=None, out_bufs=2):
                """free-layout matvec: returns sbuf AP [1, m_out] of
                act(W @ concat(segs) + b)."""
                psum = psp.tile([1, max(m_out, 1152)], f32,
                                name=f"ps_{out_name}", tag=psum_tag,
                                bufs=psum_bufs)
                chunks = load_w_chunks(wname, segs, m_out)
                nch = len(chunks)
                for ci, (wt_ap, rhs, ksz) in enumerate(chunks):
                    for n0, nsz in _nsplits(m_out):
                        nc.tensor.matmul(
                            psum[0:1, n0 : n0 + nsz],
                            rhs,
                            wt_ap[0:ksz, n0 : n0 + nsz],
                            start=(ci == 0),
                            stop=(ci == nch - 1),
                            skip_group_check=True,
                        )
                out = vecp.tile([1, m_out], f32, name=out_name,
                                tag=out_tag or out_name,
                                bufs=out_bufs if out_tag else 1)
                nc.vector.tensor_add(out, psum[0:1, 0:m_out], bias_tile)
                if act is not None:
                    nc.scalar.activation(out, out, act)
                return out

            def to_play(free_ap, d, name):
                """transpose free-layout [1, d] -> P-layout [128, ncols]."""
                n_m = _ncols(d)
                ps_t = psp.tile([128, NM2], f32, name=f"pst_{name}",
                                tag="tp", bufs=1)
                for c in range(n_m):
                    csz = min(128, d - c * 128)
                    nc.tensor.matmul(
                        ps_t[0:csz, c : c + 1],
                        free_ap[0:1, c * 128 : c * 128 + csz],
                        ident,
                        is_transpose=True,
                        start=(c == 0),
                        stop=(c == n_m - 1),
                        skip_group_check=True,
                    )
                pl = vecp.tile([128, n_m], f32r, name=name, tag=name)
                nc.vector.tensor_copy(pl, ps_t[:, 0:n_m])
                return _Vec(pl, d)

            def gru(g, x_segs, h, out_name):
                rz = matvec_f(f"wrz_{g}", x_segs + [h], 2 * H,
                              bt[f"brz_{g}"], AF.Sigmoid, f"rz_{g}",
                              "mv1", 1, out_tag="rz_sb")
                gin = matvec_f(f"win_{g}", x_segs, H, bt[f"bin_{g}"], None,
                               f"gin_{g}", "mv1", 1, out_tag="gin_sb")
                ghn = matvec_f(f"whn_{g}", [h], H, bt[f"bhn_{g}"], None,
                               f"ghn_{g}", "mv1", 1, out_tag="ghn_sb")
                # n = tanh(gin + r * ghn);  h' = n + z * (h - n)
                t3 = vecp.tile([1, H], f32, name=f"t3_{g}", tag="t3",
                                bufs=1)
                nc.vector.tensor_mul(t3, rz[0:1, 0:H], ghn)
                nc.vector.tensor_add(t3, gin, t3)
                n_t = vecp.tile([1, H], f32, name=f"n_{g}", tag="n_t",
                                bufs=1)
                nc.scalar.activation(n_t, t3, AF.Tanh)
                t5 = vecp.tile([1, H], f32, name=f"t5_{g}", tag="t5",
                                bufs=1)
                nc.vector.tensor_sub(t5, hf[g], n_t)
                nc.vector.tensor_mul(t5, rz[0:1, H : 2 * H], t5)
                hn = vecp.tile([1, H], f32, name=out_name, tag="hn",
                                bufs=1)
                nc.vector.tensor_add(hn, n_t, t5)
                return hn

            # ---- the chain ----
            out5_f = matvec_f("w5", [x5], 480, bt["b5"], AF.Relu,
                              "out5_f", "mv1", 1, out_tag="vf")
            out5 = to_play(out5_f, 480, "out5")
            _dbg("out5", out5.tile, (128, 4))
            hQ_f = gru("q", [out5], h_q, "hQ_f")
            hQ = to_play(hQ_f, H, "hQ")
            _dbg("hQ", hQ.tile, (128, 5))
            out6_f = matvec_f("w6", [x6], 480, bt["b6"], AF.Relu,
                              "out6_f", "mv1", 1, out_tag="vf")
            out6 = to_play(out6_f, 480, "out6")
            _dbg("out6", out6.tile, (128, 4))
            hSig_f = gru("sig", [hQ, out6], h_sig, "hSig_f")
            hSig = to_play(hSig_f, H, "hSig")
            _dbg("hSig", hSig.tile, (128, 5))
            out1_f = matvec_f("w1", [hSig], H, bt["b1"], AF.Relu,
                              "out1_f", "mv1", 1, out_tag="vf")
            out1 = to_play(out1_f, H, "out1")
            _dbg("out1", out1.tile, (128, 5))
            out7_f = matvec_f("w7", [obs], 960, bt["b7"], AF.Relu,
                              "out7_f", "mv1", 1, out_tag="vf")
            out7 = to_play(out7_f, 960, "out7")
            _dbg("out7", out7.tile, (128, 8))
            hS_f = gru("s", [out1, out7], h_s, "hS_f")
            if dbg:
                hS = to_play(hS_f, H, "hS")
                _dbg("hS", hS.tile, (128, 5))

            # ---- FC2a: h_fc = relu(W2a_shard @ [hSig, hS] + b2a_shard) ----
            # Build in2 = concat(hSig, hS) contiguously in free layout, then
            # transpose to a clean [128, 9] P-layout (1152 = 9*128 exactly).
            # Per output stripe of 512 the whole [1152, 512] weight block
            # arrives as ONE 2.36MB DMA.
            in2_f = vecp.tile([1, D2_IN], f32, name="in2_f", tag="in2_f")
            nc.vector.tensor_copy(in2_f[0:1, 0:H], hSig_f)
            nc.vector.tensor_copy(in2_f[0:1, H : 2 * H], hS_f)
            in2 = to_play(in2_f, D2_IN, "in2t")
            NK2 = D2_IN // 128  # 9
            ps_hfc = psp.tile([128, NM2], f32, name="ps_hfc", tag="tp",
                              bufs=1)
            n_tp = 0
            for m0, nsz in _nsplits(MSH):
                psf = psp.tile([1, STRIPE], f32, name=f"ps_f{m0}",
                               tag="fca", bufs=2)
                b2s = vecp.tile([1, STRIPE], f32, name=f"b2s_{m0}",
                                tag="b2as", bufs=2)
                nc.sync.dma_start(out=b2s[0:1, 0:nsz],
                                  in_=d_b2a[0:1, m0 : m0 + nsz])
                hstr = vecp.tile([1, STRIPE], f32, name=f"hstr_{m0}",
                                 tag="hstr", bufs=2)
                wt = bigp.tile([128, NK2, nsz], f32r, tag="w2a",
                               name=f"w2a_{m0}", bufs=3)
                nc.sync.dma_start(
                    out=wt,
                    in_=dw["w2a"][:, m0 : m0 + nsz].rearrange(
                        "(b p) m -> p b m", p=128
                    ),
                )
                rhs_cols = list(in2.chunks())
                for ci in range(NK2):
                    nc.tensor.matmul(
                        psf[0:1, 0:nsz],
                        rhs_cols[ci][0],
                        wt[:, ci, 0:nsz],
                        start=(ci == 0),
                        stop=(ci == NK2 - 1),
                        skip_group_check=True,
                    )
                # bias + relu into the free-layout accumulator
                nc.vector.tensor_add(
                    hstr[0:1, 0:nsz], psf[0:1, 0:nsz], b2s[0:1, 0:nsz]
                )
                nc.scalar.activation(
                    hstr[0:1, 0:nsz], hstr[0:1, 0:nsz], AF.Relu
                )
                # transpose this stripe into P-layout columns
                for c in range(nsz // 128):
                    col = m0 // 128 + c
                    nc.tensor.matmul(
                        ps_hfc[:, col : col + 1],
                        hstr[0:1, c * 128 : (c + 1) * 128],
                        ident,
                        is_transpose=True,
                        start=(n_tp == 0),
                        stop=(n_tp == NM2 - 1),
                        skip_group_check=True,
                    )
                    n_tp += 1
            h_fc = vecp.tile([128, NM2], f32r, name="h_fc", tag="h_fc")
            nc.vector.tensor_copy(h_fc, ps_hfc)
            _dbg("h_fc", h_fc, (128, NM2))

            # ---- FC2b: y_partial = W2b_shard @ h_fc  (out [1, 576]) ----
            ps512 = psp.tile([1, 512], f32, name="ps_y512", tag="y512",
                             bufs=1)
            ps64 = psp.tile([1, 64], f32, name="ps_y64", tag="y64", bufs=1)
            for g in range(NM2 // W2B_GRP):
                wt = w2bp.tile([128, W2B_GRP, D2_OUT], f32r, tag="w2b",
                               name=f"w2b_{g}", bufs=2)
                r0 = g * W2B_GRP * 128
                nc.sync.dma_start(
                    out=wt,
                    in_=dw["w2b"][r0 : r0 + W2B_GRP * 128, :].rearrange(
                        "(b p) m -> p b m", p=128
                    ),
                )
                for j in range(W2B_GRP):
                    kb = g * W2B_GRP + j
                    lhs = h_fc[:, kb : kb + 1]
                    nc.tensor.matmul(
                        ps512[0:1, :], lhs,
                        wt[:, j, 0:512],
                        start=(kb == 0), stop=(kb == NM2 - 1),
                        skip_group_check=True,
                    )
                    nc.tensor.matmul(
                        ps64[0:1, :], lhs,
                        wt[:, j, 512:576],
                        start=(kb == 0), stop=(kb == NM2 - 1),
                        skip_group_check=True,
                    )
            y_sb = constp.tile([1, D2_OUT], f32, name="y_sb", tag="y_sb")
            nc.vector.tensor_copy(y_sb[:, 0:512], ps512)
            nc.vector.tensor_copy(y_sb[:, 512:576], ps64)
            nc.sync.dma_start(out=d_y[:], in_=y_sb)

    nc.compile()
    return nc


def _get_program():
    if "nc" not in _CACHE:
        _CACHE["nc"] = _build_program()
    return _CACHE["nc"]


# ----------------------------------------------------------------------------
# host-side data prep
# ----------------------------------------------------------------------------


def _play(v, ncols):
    """length-d vector -> P-layout [128, ncols] (zero padded)."""
    v = np.asarray(v, F32).ravel()
    buf = np.zeros((ncols, 128), F32)
    buf.reshape(-1)[: v.size] = v
    return np.ascontiguousarray(buf.T)


def _prep_inputs(inputs):
    """Build the 8 per-core input maps from the full (unsharded) inputs."""
    g = {k: np.asarray(v, F32) for k, v in inputs.items()}

    common = {
        "x5": g["fw_evol_diff"].reshape(24, 1).copy(),
        "x6": g["fw_update_diff"].reshape(24, 1).copy(),
        "obs": np.concatenate(
            [g["obs_diff"], g["obs_innov_diff"]]
        ).reshape(48, 1).copy(),
        "h_q": _play(g["h_Q"], 5),
        "h_sig": _play(g["h_Sigma"], 5),
        "h_s": _play(g["h_S"], 5),
        "h_q_f": g["h_Q"].reshape(1, H).copy(),
        "h_sig_f": g["h_Sigma"].reshape(1, H).copy(),
        "h_s_f": g["h_S"].reshape(1, H).copy(),
        "w5": np.ascontiguousarray(g["W5"].T),
        "w6": np.ascontiguousarray(g["W6"].T),
        "w7": np.ascontiguousarray(g["W7"].T),
        "w1": np.ascontiguousarray(g["W1"].T),
        "b5": g["b5"].reshape(1, -1).copy(),
        "b6": g["b6"].reshape(1, -1).copy(),
        "b7": g["b7"].reshape(1, -1).copy(),
        "b1": g["b1"].reshape(1, -1).copy(),
    }
    for tag, suf in (("q", "Q"), ("sig", "Sig"), ("s", "S")):
        Wih, Whh = g[f"Wih_{suf}"], g[f"Whh_{suf}"]
        bih, bhh = g[f"bih_{suf}"], g[f"bhh_{suf}"]
        common[f"wrz_{tag}"] = np.ascontiguousarray(
            np.concatenate([Wih[0 : 2 * H], Whh[0 : 2 * H]], axis=1).T
        )
        common[f"win_{tag}"] = np.ascontiguousarray(Wih[2 * H :].T)
        common[f"whn_{tag}"] = np.ascontiguousarray(Whh[2 * H :].T)
        common[f"brz_{tag}"] = (bih[0 : 2 * H] + bhh[0 : 2 * H]).reshape(1, -1)
        common[f"bin_{tag}"] = bih[2 * H :].reshape(1, -1).copy()
        common[f"bhn_{tag}"] = bhh[2 * H :].reshape(1, -1).copy()

    in_maps = []
    for k in range(NCORES):
        m = dict(common)
        sl = slice(k * MSH, (k + 1) * MSH)
        m["w2a"] = np.ascontiguousarray(g["W2a"][sl, :].T)
        m["w2b"] = np.ascontiguousarray(g["W2b"][:, sl].T)
        m["b2a"] = g["b2a"][sl].reshape(1, -1).copy()
        in_maps.append(m)
    return in_maps


def run(trace=False, **inputs):
    from concourse.bass_utils import run_bass_kernel_spmd

    nc = _get_program()
    in_maps = _prep_inputs(inputs)
    res = run_bass_kernel_spmd(nc, in_maps, list(range(NCORES)), trace=trace)
    y = np.zeros(D2_OUT, np.float64)
    for r in res.results:
        y += r["y"].reshape(-1).astype(np.float64)
    out = (y.astype(F32) + np.asarray(inputs["b2b"], F32)).reshape(24, 24)
    return out, res


def kernel(**inputs):
    out, _ = run(trace=False, **inputs)
    return out



# revision 9
# speedup vs baseline: 1.8080x; 1.8080x over previous
"""Trainium2 Bass kernel for the KNet-style recurrent chain (batch=1).

Strategy (memory-bound, ~353MB fp32 weights on host):
  - ALL weights are converted to bf16 on the host: halves HBM traffic and
    the PE moving-operand time (bf16 moving = 1 cycle/row at any width).
    Host-sim rel err of bf16 weights is 2.4e-3 (gate: 2e-2).
  - The small GRU chain + small FCs are REPLICATED on all 8 cores; FC2
    (W2a [46080,1152], W2b [576,46080]) is tensor-parallel: each core
    takes 5760 rows of W2a / columns of W2b; host sums the 8 partials.
  - Every matvec y = W @ x runs weight-moving on the TensorEngine:
        psum[1, N] (+)= x_chunk[K, 1].T @ W.T_chunk[K, N]
    with the tiny fp32 activation chunk stationary and bf16 weights
    streaming as the moving operand.
  - Weights are HOST-PACKED into the exact SBUF tile layout, so every
    weight DMA is 128 contiguous rows of >=2KB: the DMA queues are
    descriptor-rate-bound near ~2KB/descriptor, so layout determines
    whether we reach the HBM roofline.
  - DMA issue order = consumption order: chain weights (deep 6-buf
    prefetch), then 12 streamed FC2a stripes (bufs=6), then 12 FC2b
    groups (bufs=2) - single issue engine, so the serial chain is never
    queued behind the big FC2 bytes.
  - FC2a output stripes are transposed to P-layout per-stripe so FC2b
    k-blocks pipeline right behind FC2a stripes.
"""

import sys

sys.path.insert(0, "/opt/trn_rl_repo")

import numpy as np
import ml_dtypes

NCORES = 8
H = 576                       # hidden size of all three GRUs
D2_HID, D2_IN, D2_OUT = 46080, 1152, 576
MSH = D2_HID // NCORES        # 5760 rows of W2a per core
NM2 = MSH // 128              # 45 h_fc columns per core
NSTR = 12                     # FC2a stripes: 11x512 + 1x128
W2B_GRP = 4                   # FC2b k-blocks per group: 45 = 11*4 + 1
CAP = 8192                    # bytes/partition per chain weight group

F32 = np.float32
BF = ml_dtypes.bfloat16

# chain weights: name -> (seg K sizes, m_out)
WSPECS = {
    "w5": ([24], 480), "w6": ([24], 480), "w7": ([48], 960),
    "w1": ([H], H),
    "wrz_q": ([480, H], 1152), "win_q": ([480], H), "whn_q": ([H], H),
    "wrz_sig": ([H, 480, H], 1152), "win_sig": ([H, 480], H),
    "whn_sig": ([H], H),
    "wrz_s": ([H, 960, H], 1152), "win_s": ([H, 960], H),
    "whn_s": ([H], H),
}
BSHAPES = {
    "b5": 480, "b6": 480, "b7": 960, "b1": H,
    "brz_q": 1152, "bin_q": H, "bhn_q": H,
    "brz_sig": 1152, "bin_sig": H, "bhn_sig": H,
    "brz_s": 1152, "bin_s": H, "bhn_s": H,
}


def _stripes():
    return [(s * 512, min(512, MSH - s * 512)) for s in range(NSTR)]


def _w2b_groups():
    return [(g * W2B_GRP, min(W2B_GRP, NM2 - g * W2B_GRP))
            for g in range((NM2 + W2B_GRP - 1) // W2B_GRP)]


def _grp(m_out):
    return max(1, CAP // (m_out * 2))


def _wplan(segs, m_out):
    """Deterministic chunk plan shared by the builder and the host packer.

    Returns (full_groups, tails, total_flat):
      full_groups: list of (seg_idx, g0, gn, flat_off)  [flat offsets in the
        packed [128, total_flat] dram tensor, units of elements]
      tails: list of (seg_idx, tail_rows)
    """
    g = _grp(m_out)
    full, tails = [], []
    off = 0
    for si, d in enumerate(segs):
        nb, tail = d // 128, d % 128
        for g0 in range(0, nb, g):
            gn = min(g, nb - g0)
            full.append((si, g0, gn, off))
            off += gn * m_out
        if tail:
            tails.append((si, tail))
    return full, tails, off


def _ncols(d):
    return (d + 127) // 128


_CACHE = {}


class _Vec:
    """An activation vector in SBUF P-layout [128, ncols]."""

    def __init__(self, tile, d):
        self.tile = tile
        self.d = d

    def chunks(self):
        for c in range(_ncols(self.d)):
            sz = min(128, self.d - c * 128)
            yield self.tile[0:sz, c : c + 1], sz


def _build_program():
    import concourse.bass as bass  # noqa: F401
    from concourse import bacc, mybir
    import concourse.tile as tile

    f32 = mybir.dt.float32
    f32r = mybir.dt.float32r
    bf16 = mybir.dt.bfloat16
    AF = mybir.ActivationFunctionType

    nc = bacc.Bacc(
        "TRN2", target_bir_lowering=False, debug=False, num_devices=NCORES
    )

    def din(name, shape, dt=f32):
        return nc.dram_tensor(name, list(shape), dt, kind="ExternalInput")

    # --- activation inputs ---
    d_x5 = din("x5", (24, 1), bf16)
    d_x6 = din("x6", (24, 1), bf16)
    d_obs = din("obs", (48, 1), bf16)
    d_hq = din("h_q", (128, 5), bf16)      # P-layout (matvec operand)
    d_hsig = din("h_sig", (128, 5), bf16)
    d_hs = din("h_s", (128, 5), bf16)
    d_hq_f = din("h_q_f", (1, H))          # free-layout (elementwise)
    d_hsig_f = din("h_sig_f", (1, H))
    d_hs_f = din("h_s_f", (1, H))

    # --- chain weights: host-packed bf16 ---
    dwf, dwt = {}, {}
    for wname, (segs, m_out) in WSPECS.items():
        full, tails, tot = _wplan(segs, m_out)
        if tot:
            dwf[wname] = din(f"{wname}_f", (128, tot), bf16)
        for si, trows in tails:
            dwt[(wname, si)] = din(f"{wname}_t{si}", (trows, m_out), bf16)

    # --- biases fp32 free-layout ---
    db = {k: din(k, (1, v)) for k, v in BSHAPES.items()}
    d_b2a = din("b2a", (1, MSH))

    # --- FC2 weights: host-packed bf16, stripe/group major ---
    d_w2a = din("w2a", (NSTR - 1, 128, 9 * 512), bf16)   # stripes 0..10
    d_w2a_t = din("w2a_t", (128, 9 * 128), bf16)         # stripe 11 (128 wide)
    d_w2b = din("w2b", (len(_w2b_groups()) - 1, 128, W2B_GRP * D2_OUT), bf16)
    d_w2b_t = din("w2b_t", (128, 1 * D2_OUT), bf16)      # last group (1 kb)

    d_y = nc.dram_tensor("y", [1, D2_OUT], f32, kind="ExternalOutput")

    with tile.TileContext(nc) as tc:
        with (
            tc.tile_pool(name="const", bufs=1) as constp,
            tc.tile_pool(name="vecs", bufs=1) as vecp,
            tc.tile_pool(name="biasp", bufs=1) as biasp,
            tc.tile_pool(name="smallw", bufs=1) as swp,
            tc.tile_pool(name="w2ap", bufs=1) as w2ap,
            tc.tile_pool(name="w2bp", bufs=1) as w2bp,
            tc.tile_pool(name="ps", bufs=1, space="PSUM") as psp,
        ):
            def load_const(dram, shape, name, dt=f32):
                t = constp.tile(list(shape), dt, name=name, tag=name)
                nc.sync.dma_start(out=t, in_=dram[:])
                return t

            x5 = _Vec(load_const(d_x5, (24, 1), "t_x5", bf16), 24)
            x6 = _Vec(load_const(d_x6, (24, 1), "t_x6", bf16), 24)
            obs = _Vec(load_const(d_obs, (48, 1), "t_obs", bf16), 48)
            h_q = _Vec(load_const(d_hq, (128, 5), "t_hq", bf16), H)
            h_sig = _Vec(load_const(d_hsig, (128, 5), "t_hsig", bf16), H)
            h_s = _Vec(load_const(d_hs, (128, 5), "t_hs", bf16), H)
            hf = {
                "q": load_const(d_hq_f, (1, H), "t_hq_f"),
                "sig": load_const(d_hsig_f, (1, H), "t_hsig_f"),
                "s": load_const(d_hs_f, (1, H), "t_hs_f"),
            }
            b2a_sb = load_const(d_b2a, (1, MSH), "t_b2a")
            ident = constp.tile([1, 1], f32, name="ident", tag="ident")
            nc.vector.memset(ident, 1.0)
            h_fc = constp.tile([128, NM2], bf16, name="h_fc", tag="h_fc")

            def matvec_f(wname, segs, m_out, act, out_name, out_tag,
                        out_bufs=1):
                """free-layout matvec: sbuf [1, m_out] of
                act(W @ concat(segs) + b)."""
                bias = biasp.tile([1, m_out], f32, name=f"b_{out_name}",
                                  tag="bias", bufs=2)
                nc.sync.dma_start(out=bias, in_=db["b" + wname[1:]][:])
                full, tails, _ = _wplan([v.d for v in segs], m_out)
                chunks = []          # (wt_ap, rhs_ap, ksz)
                seg_cols = [list(v.chunks()) for v in segs]
                for si, g0, gn, off in full:
                    wt = swp.tile([128, gn, m_out], bf16, tag="sw",
                                  name=f"w_{wname}_{si}_{g0}", bufs=5)
                    nc.sync.dma_start(
                        out=wt,
                        in_=dwf[wname][:, off : off + gn * m_out],
                    )
                    for b in range(gn):
                        chunks.append((wt[:, b, :], seg_cols[si][g0 + b][0],
                                       128))
                for si, trows in tails:
                    nb = segs[si].d // 128
                    wtt = swp.tile([trows, m_out], bf16, tag="sw",
                                   name=f"w_{wname}_t{si}", bufs=5)
                    nc.sync.dma_start(out=wtt, in_=dwt[(wname, si)][:])
                    chunks.append((wtt, seg_cols[si][nb][0], trows))

                psum = psp.tile([1, max(m_out, 1152)], f32,
                                name=f"ps_{out_name}", tag="mv1", bufs=1)
                nch = len(chunks)
                nsplits = [(n0, min(512, m_out - n0))
                           for n0 in range(0, m_out, 512)]
                for ci, (wt_ap, rhs, ksz) in enumerate(chunks):
                    for n0, nsz in nsplits:
                        nc.tensor.matmul(
                            psum[0:1, n0 : n0 + nsz],
                            rhs,
                            wt_ap[0:ksz, n0 : n0 + nsz],
                            start=(ci == 0),
                            stop=(ci == nch - 1),
                            skip_group_check=True,
                        )
                out = vecp.tile([1, m_out], f32, name=out_name,
                                tag=out_tag, bufs=out_bufs)
                nc.vector.tensor_add(out, psum[0:1, 0:m_out], bias)
                if act is not None:
                    nc.scalar.activation(out, out, act)
                return out

            def to_play(free_ap, d, name):
                """transpose free-layout [1, d] -> P-layout [128, ncols]."""
                n_m = _ncols(d)
                ps_t = psp.tile([128, 9], f32, name=f"pst_{name}",
                                tag="tp", bufs=1)
                for c in range(n_m):
                    csz = min(128, d - c * 128)
                    nc.tensor.matmul(
                        ps_t[0:csz, c : c + 1],
                        free_ap[0:1, c * 128 : c * 128 + csz],
                        ident,
                        is_transpose=True,
                        start=(c == 0),
                        stop=(c == n_m - 1),
                        skip_group_check=True,
                    )
                pl = vecp.tile([128, n_m], bf16, name=name, tag=name)
                nc.vector.tensor_copy(pl, ps_t[:, 0:n_m])
                return _Vec(pl, d)

            def gru(g, x_segs, h, out_name):
                rz = matvec_f(f"wrz_{g}", x_segs + [h], 2 * H, AF.Sigmoid,
                              f"rz_{g}", "rz")
                gin = matvec_f(f"win_{g}", x_segs, H, None, f"gin_{g}",
                               "gv", 2)
                ghn = matvec_f(f"whn_{g}", [h], H, None, f"ghn_{g}",
                               "gv", 2)
                # n = tanh(gin + r * ghn);  h' = n + z * (h - n)
                t3 = vecp.tile([1, H], f32, name=f"t3_{g}", tag="t3")
                nc.vector.tensor_mul(t3, rz[0:1, 0:H], ghn)
                nc.vector.tensor_add(t3, gin, t3)
                n_t = vecp.tile([1, H], f32, name=f"n_{g}", tag="n_t")
                nc.scalar.activation(n_t, t3, AF.Tanh)
                t5 = vecp.tile([1, H], f32, name=f"t5_{g}", tag="t5")
                nc.vector.tensor_sub(t5, hf[g], n_t)
                nc.vector.tensor_mul(t5, rz[0:1, H : 2 * H], t5)
                hn = vecp.tile([1, H], f32, name=out_name, tag=out_name)
                nc.vector.tensor_add(hn, n_t, t5)
                return hn

            # ---- the serial chain ----
            out5_f = matvec_f("w5", [x5], 480, AF.Relu, "out5_f", "vf", 2)
            out5 = to_play(out5_f, 480, "out5")
            hQ_f = gru("q", [out5], h_q, "hQ_f")
            hQ = to_play(hQ_f, H, "hQ")
            out6_f = matvec_f("w6", [x6], 480, AF.Relu, "out6_f", "vf", 2)
            out6 = to_play(out6_f, 480, "out6")
            hSig_f = gru("sig", [hQ, out6], h_sig, "hSig_f")
            hSig = to_play(hSig_f, H, "hSig")
            out1_f = matvec_f("w1", [hSig], H, AF.Relu, "out1_f", "vf", 2)
            out1 = to_play(out1_f, H, "out1")
            out7_f = matvec_f("w7", [obs], 960, AF.Relu, "out7_f", "vf", 2)
            out7 = to_play(out7_f, 960, "out7")
            hS_f = gru("s", [out1, out7], h_s, "hS_f")

            # in2 = concat(hSig, hS) -> P-layout [128, 9]
            in2_f = vecp.tile([1, D2_IN], f32, name="in2_f", tag="in2_f")
            nc.vector.tensor_copy(in2_f[0:1, 0:H], hSig_f)
            nc.vector.tensor_copy(in2_f[0:1, H : 2 * H], hS_f)
            in2 = to_play(in2_f, D2_IN, "in2t")
            rhs_cols = list(in2.chunks())

            # ---- FC2a weight stream: issue all stripe DMAs now ----
            w2a_tiles = []
            for s, (m0, nsz) in enumerate(_stripes()):
                wt = w2ap.tile([128, 9, nsz], bf16, tag="w2a",
                               name=f"w2a_{s}", bufs=5)
                src = d_w2a[s] if nsz == 512 else d_w2a_t
                nc.sync.dma_start(out=wt, in_=src[:])
                w2a_tiles.append(wt)

            # ---- FC2a stripes + pipelined FC2b ----
            ps_y512 = psp.tile([1, 512], f32, name="ps_y512", tag="y512",
                               bufs=1)
            ps_y64 = psp.tile([1, 64], f32, name="ps_y64", tag="y64",
                              bufs=1)
            groups = _w2b_groups()
            w2b_done = 0

            for s, (m0, nsz) in enumerate(_stripes()):
                wt = w2a_tiles[s]
                psf = psp.tile([1, 512], f32, name=f"ps_f{s}", tag="fca",
                               bufs=2)
                for ci in range(9):
                    nc.tensor.matmul(
                        psf[0:1, 0:nsz],
                        rhs_cols[ci][0],
                        wt[0:128, ci, 0:nsz],
                        start=(ci == 0),
                        stop=(ci == 8),
                        skip_group_check=True,
                    )
                hstr = vecp.tile([1, 512], f32, name=f"hstr_{s}",
                                 tag="hstr", bufs=2)
                nc.vector.tensor_add(
                    hstr[0:1, 0:nsz], psf[0:1, 0:nsz],
                    b2a_sb[0:1, m0 : m0 + nsz]
                )
                nc.scalar.activation(
                    hstr[0:1, 0:nsz], hstr[0:1, 0:nsz], AF.Relu
                )
                # transpose stripe into h_fc P-layout columns
                ncol = nsz // 128
                ps_t = psp.tile([128, 9], f32, name=f"pst_s{s}", tag="tp",
                                bufs=1)
                for c in range(ncol):
                    nc.tensor.matmul(
                        ps_t[:, c : c + 1],
                        hstr[0:1, c * 128 : (c + 1) * 128],
                        ident,
                        is_transpose=True,
                        start=(c == 0),
                        stop=(c == ncol - 1),
                        skip_group_check=True,
                    )
                col0 = m0 // 128
                nc.vector.tensor_copy(
                    h_fc[:, col0 : col0 + ncol], ps_t[:, 0:ncol]
                )
                # FC2b: issue group DMAs + matmuls for every group whose
                # h_fc columns are now complete
                cols_done = col0 + ncol
                while (w2b_done < len(groups)
                       and groups[w2b_done][0] + groups[w2b_done][1]
                       <= cols_done):
                    g, (kb0, kn) = w2b_done, groups[w2b_done]
                    wtb = w2bp.tile([128, kn, D2_OUT], bf16, tag="w2b",
                                    name=f"w2b_{g}", bufs=2)
                    src = d_w2b[g] if kn == W2B_GRP else d_w2b_t
                    nc.sync.dma_start(out=wtb, in_=src[:])
                    for j in range(kn):
                        kb = kb0 + j
                        lhs = h_fc[:, kb : kb + 1]
                        nc.tensor.matmul(
                            ps_y512[0:1, :], lhs,
                            wtb[0:128, j, 0:512],
                            start=(kb == 0), stop=(kb == NM2 - 1),
                            skip_group_check=True,
                        )
                        nc.tensor.matmul(
                            ps_y64[0:1, :], lhs,
                            wtb[0:128, j, 512:576],
                            start=(kb == 0), stop=(kb == NM2 - 1),
                            skip_group_check=True,
                        )
                    w2b_done += 1

            y_sb = constp.tile([1, D2_OUT], f32, name="y_sb", tag="y_sb")
            nc.vector.tensor_copy(y_sb[:, 0:512], ps_y512)
            nc.vector.tensor_copy(y_sb[:, 512:576], ps_y64)
            nc.sync.dma_start(out=d_y[:], in_=y_sb)

    nc.compile()
    return nc


def _get_program():
    if "nc" not in _CACHE:
        _CACHE["nc"] = _build_program()
    return _CACHE["nc"]


# ----------------------------------------------------------------------------
# host-side data prep
# ----------------------------------------------------------------------------


def _play(v, ncols):
    """length-d vector -> P-layout [128, ncols] bf16 (zero padded)."""
    v = np.asarray(v, F32).ravel()
    buf = np.zeros((ncols, 128), F32)
    buf.reshape(-1)[: v.size] = v
    return np.ascontiguousarray(buf.T).astype(BF)


def _pack_w(wt, segs, m_out):
    """Pack W.T [K, m_out] (fp32) into (flat [128, tot] bf16,
    {seg_idx: tail [trows, m_out] bf16}) mirroring _wplan."""
    full, tails, tot = _wplan(segs, m_out)
    wt = np.asarray(wt, F32)
    flat = np.empty((128, tot), BF) if tot else None
    seg_off = np.concatenate([[0], np.cumsum(segs)]).astype(int)
    for si, g0, gn, off in full:
        ro = seg_off[si] + g0 * 128
        blk = wt[ro : ro + gn * 128].reshape(gn, 128, m_out)
        flat[:, off : off + gn * m_out] = (
            blk.transpose(1, 0, 2).reshape(128, gn * m_out).astype(BF)
        )
    tail_arrs = {}
    for si, trows in tails:
        ro = seg_off[si] + (segs[si] // 128) * 128
        tail_arrs[si] = np.ascontiguousarray(wt[ro : ro + trows]).astype(BF)
    return flat, tail_arrs


def _prep_inputs(inputs):
    """Build the 8 per-core input maps from the full (unsharded) inputs."""
    g = {k: np.asarray(v, F32) for k, v in inputs.items()}

    common = {
        "x5": g["fw_evol_diff"].reshape(24, 1).astype(BF),
        "x6": g["fw_update_diff"].reshape(24, 1).astype(BF),
        "obs": np.concatenate(
            [g["obs_diff"], g["obs_innov_diff"]]
        ).reshape(48, 1).astype(BF),
        "h_q": _play(g["h_Q"], 5),
        "h_sig": _play(g["h_Sigma"], 5),
        "h_s": _play(g["h_S"], 5),
        "h_q_f": g["h_Q"].reshape(1, H).copy(),
        "h_sig_f": g["h_Sigma"].reshape(1, H).copy(),
        "h_s_f": g["h_S"].reshape(1, H).copy(),
        "b5": g["b5"].reshape(1, -1).copy(),
        "b6": g["b6"].reshape(1, -1).copy(),
        "b7": g["b7"].reshape(1, -1).copy(),
        "b1": g["b1"].reshape(1, -1).copy(),
    }

    wT = {
        "w5": g["W5"].T, "w6": g["W6"].T, "w7": g["W7"].T, "w1": g["W1"].T,
    }
    for tag, suf in (("q", "Q"), ("sig", "Sig"), ("s", "S")):
        Wih, Whh = g[f"Wih_{suf}"], g[f"Whh_{suf}"]
        bih, bhh = g[f"bih_{suf}"], g[f"bhh_{suf}"]
        wT[f"wrz_{tag}"] = np.concatenate(
            [Wih[0 : 2 * H], Whh[0 : 2 * H]], axis=1
        ).T
        wT[f"win_{tag}"] = Wih[2 * H :].T
        wT[f"whn_{tag}"] = Whh[2 * H :].T
        common[f"brz_{tag}"] = (bih[0 : 2 * H] + bhh[0 : 2 * H]).reshape(1, -1)
        common[f"bin_{tag}"] = bih[2 * H :].reshape(1, -1).copy()
        common[f"bhn_{tag}"] = bhh[2 * H :].reshape(1, -1).copy()

    for wname, (segs, m_out) in WSPECS.items():
        flat, tails = _pack_w(wT[wname], segs, m_out)
        if flat is not None:
            common[f"{wname}_f"] = flat
        for si, arr in tails.items():
            common[f"{wname}_t{si}"] = arr

    stripes = _stripes()
    groups = _w2b_groups()
    in_maps = []
    for k in range(NCORES):
        m = dict(common)
        sl = slice(k * MSH, (k + 1) * MSH)
        w2aT = np.ascontiguousarray(g["W2a"][sl, :].T)       # [1152, 5760]
        w2aP = w2aT.reshape(9, 128, MSH).transpose(1, 0, 2)  # [128, 9, 5760]
        w2a_full = np.empty((NSTR - 1, 128, 9 * 512), BF)
        for s, (m0, nsz) in enumerate(stripes[:-1]):
            w2a_full[s] = (
                w2aP[:, :, m0 : m0 + nsz].reshape(128, 9 * 512).astype(BF)
            )
        m0, nsz = stripes[-1]
        m["w2a"] = w2a_full
        m["w2a_t"] = np.ascontiguousarray(
            w2aP[:, :, m0 : m0 + nsz].reshape(128, 9 * nsz)
        ).astype(BF)

        w2bT = np.ascontiguousarray(g["W2b"][:, sl].T)       # [5760, 576]
        w2bG = w2bT.reshape(NM2, 128, D2_OUT)
        w2b_full = np.empty((len(groups) - 1, 128, W2B_GRP * D2_OUT), BF)
        for gi, (kb0, kn) in enumerate(groups[:-1]):
            w2b_full[gi] = (
                w2bG[kb0 : kb0 + kn].transpose(1, 0, 2)
                .reshape(128, kn * D2_OUT).astype(BF)
            )
        kb0, kn = groups[-1]
        m["w2b"] = w2b_full
        m["w2b_t"] = np.ascontiguousarray(
            w2bG[kb0 : kb0 + kn].transpose(1, 0, 2).reshape(128, kn * D2_OUT)
        ).astype(BF)

        m["b2a"] = g["b2a"][sl].reshape(1, -1).copy()
        in_maps.append(m)
    return in_maps


def run(trace=False, **inputs):
    from concourse.bass_utils import run_bass_kernel_spmd

    nc = _get_program()
    in_maps = _prep_inputs(inputs)
    res = run_bass_kernel_spmd(nc, in_maps, list(range(NCORES)), trace=trace)
    y = np.zeros(D2_OUT, np.float64)
    for r in res.results:
        y += r["y"].reshape(-1).astype(np.float64)
    out = (y.astype(F32) + np.asarray(inputs["b2b"], F32)).reshape(24, 24)
    return out, res


def kernel(**inputs):
    out, _ = run(trace=False, **inputs)
    return out
